# revision 3
# baseline (speedup 1.0000x reference)
"""Trainium2 Bass kernel for nn_LinearFlowModel (dense_mlp), v6.

Algorithm: host pre-transposes state to fp16 stateT[d, b]; W halves are the
PE-stationary operands; stateT streams 512 batch cols per matmul; PSUM is
evacuated to fp16 SBUF (pure dtype-converting copy -- the bias is added on
the host) and DMAed out transposed; host adds bias and transposes back.
All HBM traffic is fp16: 4 MB in + 8 MB out per core (vs 24 MB fp32), which
is HBM-wall-bound at ~33 us/core; compute hides under it.

Pipeline structure (evolved v2->v6 from traces):
  - v4 dependency shape: per 1024-col group, delta half (2 matmuls -> 2-bank
    PSUM tile -> ScalarE copy -> yd) and var half (-> VectorE copy -> yv) are
    independent chains; PSUM pool bufs=4 keeps the PE a full group ahead of
    the evacuation engines. A few var halves are reassigned to ScalarE to
    balance engine time (ACT 1.2 GHz vs DVE 0.96 GHz).
  - out-DMAs all issue from the Sync sequencer: HWDGE descriptor-gen blocks
    its sequencer on the producer's completion, which on Sync idles nobody
    (v3/v4 lesson). Input chunks alternate rings and are issued up front
    with no waits; the Scalar sequencer then runs pure ACT evacuation.
  - PE warmup: dummy matmuls during the input-DMA dead window flip the HAM
    clock gate (1.2 -> 2.4 GHz) so real matmuls start warm (v5 lesson), and
    steady-state PE gaps are too short for it to re-throttle.
  - the last output block issues pair-granular DMAs to shorten the drain
    tail.
"""

import os
import sys

if "/opt/trn_rl_repo" not in sys.path:
    sys.path.insert(0, "/opt/trn_rl_repo")

import numpy as np

B = 131072
D = 128
NCORES = 8
BLOC = B // NCORES  # 16384 batch columns per core

IN_BLK = int(os.environ.get("KV6_IN_BLK", "2048"))  # input DMA chunk (0.5 MB)
CB = 1024  # per-half compute block: 2 PSUM banks
OUT_BLK = int(os.environ.get("KV6_OUT_BLK", "2048"))  # per-half out DMA chunk
YBUFS = int(os.environ.get("KV6_YBUFS", "5"))
WARM_MM = int(os.environ.get("KV6_WARM_MM", "12"))  # PE warmup matmuls
# var-half groups (of 16) whose evacuation moves DVE -> ACT for balance
ACT_EXTRA = {5, 13}

assert BLOC % IN_BLK == 0 and BLOC % OUT_BLK == 0 and OUT_BLK % CB == 0

_prog = None


def _build_program():
    import concourse.bacc as bacc
    import concourse.mybir as mybir
    from concourse import tile

    f32 = mybir.dt.float32
    f16 = mybir.dt.float16

    nc = bacc.Bacc(
        "TRN2",
        target_bir_lowering=False,
        debug=False,
        num_devices=NCORES,
    )

    xT_d = nc.dram_tensor("xT", [D, BLOC], f16, kind="ExternalInput").ap()
    w2_d = nc.dram_tensor("w2", [D, 2, D], f16, kind="ExternalInput").ap()
    dT_d = nc.dram_tensor("dT", [D, BLOC], f16, kind="ExternalOutput").ap()
    vT_d = nc.dram_tensor("vT", [D, BLOC], f16, kind="ExternalOutput").ap()

    n_in = BLOC // IN_BLK

    with tile.TileContext(nc) as tc:
        with (
            tc.tile_pool(name="const", bufs=1) as cpool,
            tc.tile_pool(name="xin", bufs=n_in) as xpool,
            tc.tile_pool(name="yd", bufs=YBUFS) as ydpool,
            tc.tile_pool(name="yv", bufs=YBUFS) as yvpool,
            tc.tile_pool(name="ps", bufs=4, space="PSUM") as pspool,
        ):
            # PE warmup operands: zeroed dummy tile (GpSimd is otherwise idle)
            dummy = cpool.tile([D, 512], f16)
            if WARM_MM:
                nc.gpsimd.memset(dummy[:], 0.0)

            xts = []
            x0 = xpool.tile([D, IN_BLK], f16, tag="x")
            nc.sync.dma_start(x0[:], xT_d[:, 0:IN_BLK])
            xts.append(x0)
            w_sb = cpool.tile([D, 2, D], f16)
            nc.scalar.dma_start(w_sb[:], w2_d[:])
            for ib in range(1, n_in):
                x = xpool.tile([D, IN_BLK], f16, tag="x")
                eng = nc.scalar if ib % 2 else nc.sync
                eng.dma_start(x[:], xT_d[:, ib * IN_BLK : (ib + 1) * IN_BLK])
                xts.append(x)

            if WARM_MM:
                warm_ps = pspool.tile([D, CB], f32, tag="ps")
                for _ in range(WARM_MM):
                    nc.tensor.matmul(
                        warm_ps[:, 0:512],
                        dummy[:, 0:D],
                        dummy[:],
                        start=True,
                        stop=True,
                    )

            for ob in range(BLOC // OUT_BLK):
                yd = ydpool.tile([D, OUT_BLK], f16, tag="yd")
                yv = yvpool.tile([D, OUT_BLK], f16, tag="yv")
                last_ob = ob == BLOC // OUT_BLK - 1
                for cb in range(OUT_BLK // CB):
                    g = ob * (OUT_BLK // CB) + cb  # global 1024-col group idx
                    boff = g * CB
                    yc = cb * CB
                    for o in range(2):
                        ps = pspool.tile([D, CB], f32, tag="ps")
                        for k in range(CB // 512):
                            goff = boff + k * 512
                            x = xts[goff // IN_BLK]
                            xo = goff % IN_BLK
                            nc.tensor.matmul(
                                ps[:, k * 512 : (k + 1) * 512],
                                w_sb[:, o, :],
                                x[:, xo : xo + 512],
                                start=True,
                                stop=True,
                            )
                        if o == 0:
                            nc.scalar.copy(yd[:, yc : yc + CB], ps[:])
                        elif g in ACT_EXTRA:
                            nc.scalar.copy(yv[:, yc : yc + CB], ps[:])
                        else:
                            nc.vector.tensor_copy(yv[:, yc : yc + CB], ps[:])
                    if last_ob:
                        # pair-granular DMAs on the (by now idle) Scalar ring:
                        # the final bytes bypass the deep Sync-ring per-engine
                        # FIFOs, so a straggler DMA engine can't add its
                        # accumulated lag to the kernel tail.
                        gc = boff
                        nc.scalar.dma_start(dT_d[:, gc : gc + CB], yd[:, yc : yc + CB])
                        nc.scalar.dma_start(vT_d[:, gc : gc + CB], yv[:, yc : yc + CB])
                if not last_ob:
                    off = ob * OUT_BLK
                    nc.sync.dma_start(dT_d[:, off : off + OUT_BLK], yd[:])
                    nc.sync.dma_start(vT_d[:, off : off + OUT_BLK], yv[:])

    nc.compile()
    return nc


def _get_program():
    global _prog
    if _prog is None:
        _prog = _build_program()
    return _prog


def _prep_inputs(state, W, b):
    state = np.asarray(state, dtype=np.float32)
    W = np.asarray(W, dtype=np.float32)
    w2 = np.ascontiguousarray(W.transpose(2, 1, 0), dtype=np.float16)  # [d, o, n]
    state16 = state.astype(np.float16)
    in_maps = []
    for i in range(NCORES):
        xT = np.ascontiguousarray(state16[i * BLOC : (i + 1) * BLOC, :].T)
        in_maps.append({"xT": xT, "w2": w2})
    return in_maps


def run_on_device(state, W, b, trace=False, **kw):
    """Run the Bass kernel on the 8 NeuronCores; returns (delta, var, results)."""
    from concourse.bass_utils import run_bass_kernel_spmd

    nc = _get_program()
    in_maps = _prep_inputs(state, W, b)
    res = run_bass_kernel_spmd(nc, in_maps, list(range(NCORES)), trace=trace, **kw)
    b = np.asarray(b, dtype=np.float32)
    delta = np.empty((B, D), dtype=np.float32)
    var = np.empty((B, D), dtype=np.float32)
    for i, r in enumerate(res.results):
        delta[i * BLOC : (i + 1) * BLOC] = r["dT"].T
        var[i * BLOC : (i + 1) * BLOC] = r["vT"].T
    delta += b[None, :, 0]
    var += b[None, :, 1]
    return delta, var, res


def kernel(state, W, b):
    try:
        delta, var, _ = run_on_device(state, W, b, trace=False)
    except Exception:
        delta, var, _ = run_on_device(state, W, b, trace=False)
    return delta, var


# revision 4
# speedup vs baseline: 1.1577x; 1.1577x over previous
"""Trainium2 Bass kernel for nn_LinearFlowModel (dense_mlp), v6.

Algorithm: host pre-transposes state to fp16 stateT[d, b]; W halves are the
PE-stationary operands; stateT streams 512 batch cols per matmul; PSUM is
evacuated to fp16 SBUF (pure dtype-converting copy -- the bias is added on
the host) and DMAed out transposed; host adds bias and transposes back.
All HBM traffic is fp16: 4 MB in + 8 MB out per core (vs 24 MB fp32), which
is HBM-wall-bound at ~33 us/core; compute hides under it.

Pipeline structure (evolved v2->v6 from traces):
  - v4 dependency shape: per 1024-col group, delta half (2 matmuls -> 2-bank
    PSUM tile -> ScalarE copy -> yd) and var half (-> VectorE copy -> yv) are
    independent chains; PSUM pool bufs=4 keeps the PE a full group ahead of
    the evacuation engines. A few var halves are reassigned to ScalarE to
    balance engine time (ACT 1.2 GHz vs DVE 0.96 GHz).
  - out-DMAs all issue from the Sync sequencer: HWDGE descriptor-gen blocks
    its sequencer on the producer's completion, which on Sync idles nobody
    (v3/v4 lesson). Input chunks alternate rings and are issued up front
    with no waits; the Scalar sequencer then runs pure ACT evacuation.
  - PE warmup: dummy matmuls during the input-DMA dead window flip the HAM
    clock gate (1.2 -> 2.4 GHz) so real matmuls start warm (v5 lesson), and
    steady-state PE gaps are too short for it to re-throttle.
  - the last output block issues pair-granular DMAs to shorten the drain
    tail.
"""

import os
import sys

if "/opt/trn_rl_repo" not in sys.path:
    sys.path.insert(0, "/opt/trn_rl_repo")

import numpy as np

B = 131072
D = 128
NCORES = 8
BLOC = B // NCORES  # 16384 batch columns per core

IN_BLK = int(os.environ.get("KV6_IN_BLK", "2048"))  # input DMA chunk (0.5 MB)
CB = 1024  # per-half compute block: 2 PSUM banks
OUT_BLK = int(os.environ.get("KV6_OUT_BLK", "2048"))  # per-half out DMA chunk
# 8 output buffers per half (64 KB/partition for both pools): deep enough to
# absorb stretched DMA completion receipts in chip-contended phases without
# starving the evacuation engines; no fast-mode cost (SBUF has room).
YBUFS = int(os.environ.get("KV6_YBUFS", "8"))
WARM_MM = int(os.environ.get("KV6_WARM_MM", "12"))  # PE warmup matmuls
# var-half groups (of 16) whose evacuation moves DVE -> ACT for balance
ACT_EXTRA = {5, 13}

assert BLOC % IN_BLK == 0 and BLOC % OUT_BLK == 0 and OUT_BLK % CB == 0

_prog = None


def _build_program():
    import concourse.bacc as bacc
    import concourse.mybir as mybir
    from concourse import tile

    f32 = mybir.dt.float32
    f16 = mybir.dt.float16

    nc = bacc.Bacc(
        "TRN2",
        target_bir_lowering=False,
        debug=False,
        num_devices=NCORES,
    )

    xT_d = nc.dram_tensor("xT", [D, BLOC], f16, kind="ExternalInput").ap()
    w2_d = nc.dram_tensor("w2", [D, 2, D], f16, kind="ExternalInput").ap()
    dT_d = nc.dram_tensor("dT", [D, BLOC], f16, kind="ExternalOutput").ap()
    vT_d = nc.dram_tensor("vT", [D, BLOC], f16, kind="ExternalOutput").ap()

    n_in = BLOC // IN_BLK

    with tile.TileContext(nc) as tc:
        with (
            tc.tile_pool(name="const", bufs=1) as cpool,
            tc.tile_pool(name="xin", bufs=n_in) as xpool,
            tc.tile_pool(name="yd", bufs=YBUFS) as ydpool,
            tc.tile_pool(name="yv", bufs=YBUFS) as yvpool,
            tc.tile_pool(name="ps", bufs=4, space="PSUM") as pspool,
        ):
            # PE warmup operands: zeroed dummy tile (GpSimd is otherwise idle)
            dummy = cpool.tile([D, 512], f16)
            if WARM_MM:
                nc.gpsimd.memset(dummy[:], 0.0)

            xts = []
            x0 = xpool.tile([D, IN_BLK], f16, tag="x")
            nc.sync.dma_start(x0[:], xT_d[:, 0:IN_BLK])
            xts.append(x0)
            w_sb = cpool.tile([D, 2, D], f16)
            nc.scalar.dma_start(w_sb[:], w2_d[:])
            for ib in range(1, n_in):
                x = xpool.tile([D, IN_BLK], f16, tag="x")
                eng = nc.scalar if ib % 2 else nc.sync
                eng.dma_start(x[:], xT_d[:, ib * IN_BLK : (ib + 1) * IN_BLK])
                xts.append(x)

            if WARM_MM:
                warm_ps = pspool.tile([D, CB], f32, tag="ps")
                for _ in range(WARM_MM):
                    nc.tensor.matmul(
                        warm_ps[:, 0:512],
                        dummy[:, 0:D],
                        dummy[:],
                        start=True,
                        stop=True,
                    )

            for ob in range(BLOC // OUT_BLK):
                yd = ydpool.tile([D, OUT_BLK], f16, tag="yd")
                yv = yvpool.tile([D, OUT_BLK], f16, tag="yv")
                last_ob = ob == BLOC // OUT_BLK - 1
                for cb in range(OUT_BLK // CB):
                    g = ob * (OUT_BLK // CB) + cb  # global 1024-col group idx
                    boff = g * CB
                    yc = cb * CB
                    for o in range(2):
                        ps = pspool.tile([D, CB], f32, tag="ps")
                        for k in range(CB // 512):
                            goff = boff + k * 512
                            x = xts[goff // IN_BLK]
                            xo = goff % IN_BLK
                            nc.tensor.matmul(
                                ps[:, k * 512 : (k + 1) * 512],
                                w_sb[:, o, :],
                                x[:, xo : xo + 512],
                                start=True,
                                stop=True,
                            )
                        if o == 0:
                            nc.scalar.copy(yd[:, yc : yc + CB], ps[:])
                        elif g in ACT_EXTRA:
                            nc.scalar.copy(yv[:, yc : yc + CB], ps[:])
                        else:
                            nc.vector.tensor_copy(yv[:, yc : yc + CB], ps[:])
                    if last_ob:
                        # pair-granular DMAs on the (by now idle) Scalar ring:
                        # the final bytes bypass the deep Sync-ring per-engine
                        # FIFOs, so a straggler DMA engine can't add its
                        # accumulated lag to the kernel tail.
                        gc = boff
                        nc.scalar.dma_start(dT_d[:, gc : gc + CB], yd[:, yc : yc + CB])
                        nc.scalar.dma_start(vT_d[:, gc : gc + CB], yv[:, yc : yc + CB])
                if not last_ob:
                    off = ob * OUT_BLK
                    nc.sync.dma_start(dT_d[:, off : off + OUT_BLK], yd[:])
                    nc.sync.dma_start(vT_d[:, off : off + OUT_BLK], yv[:])

    nc.compile()
    return nc


def _get_program():
    global _prog
    if _prog is None:
        _prog = _build_program()
    return _prog


def _prep_inputs(state, W, b):
    state = np.asarray(state, dtype=np.float32)
    W = np.asarray(W, dtype=np.float32)
    w2 = np.ascontiguousarray(W.transpose(2, 1, 0), dtype=np.float16)  # [d, o, n]
    state16 = state.astype(np.float16)
    in_maps = []
    for i in range(NCORES):
        xT = np.ascontiguousarray(state16[i * BLOC : (i + 1) * BLOC, :].T)
        in_maps.append({"xT": xT, "w2": w2})
    return in_maps


def run_on_device(state, W, b, trace=False, **kw):
    """Run the Bass kernel on the 8 NeuronCores; returns (delta, var, results)."""
    from concourse.bass_utils import run_bass_kernel_spmd

    nc = _get_program()
    in_maps = _prep_inputs(state, W, b)
    res = run_bass_kernel_spmd(nc, in_maps, list(range(NCORES)), trace=trace, **kw)
    b = np.asarray(b, dtype=np.float32)
    delta = np.empty((B, D), dtype=np.float32)
    var = np.empty((B, D), dtype=np.float32)
    for i, r in enumerate(res.results):
        delta[i * BLOC : (i + 1) * BLOC] = r["dT"].T
        var[i * BLOC : (i + 1) * BLOC] = r["vT"].T
    delta += b[None, :, 0]
    var += b[None, :, 1]
    return delta, var, res


def kernel(state, W, b):
    try:
        delta, var, _ = run_on_device(state, W, b, trace=False)
    except Exception:
        delta, var, _ = run_on_device(state, W, b, trace=False)
    return delta, var
